# revision 25
# baseline (speedup 1.0000x reference)
"""Trainium2 Bass kernel for nn_Block_343597384085.

Model (per batch b):
  c        = silu(causal_depthwise_conv(x, K=4) + conv_b)
  out_gate = silu(x @ gate_w + gate_b)
  v = ctx = out = c
  for i in 0..3:
      cn      = rmsnorm(ctx) * rms_w[i]
      alphas  = sigmoid(cn @ alpha_w[i] + alpha_b[i])
      betas   = silu(cn @ beta_w[i] + beta_b[i])
      ws      = sqrt(clip(1 - alphas^2, 1e-6))
      fetched = assoc_scan(h_t = a_t h_{t-1} + v_t) over (v*betas*ws, alphas)
      ctx     = ctx + silu(fetched @ ctx_w[i] + ctx_b[i])
      out     = out + fetched
  out = rmsnorm(out * out_gate) * fin_rms_w
  y   = silu(out @ fin_w + fin_b)

Sharding: 8 cores = (batch, seq-half). Each core: 1024 tokens x D=1024,
feature-major SBUF layout [D-block(128 part), tokens(free)].
All matmuls run in bf16 (weights converted on host; 1 cyc/row, same PE
speed as f32r, half the SBUF/DMA). The scan stays fp32. The scan's
cross-half carry moves via a per-iteration pair AllGather (4KB); the
cumprod of alphas is computed BEFORE the carry arrives (initial=1), so
post-carry work is one scalar_tensor_tensor per block.
"""
import numpy as np
import ml_dtypes

import concourse.bass as bass
import concourse.bacc as bacc
import concourse.mybir as mybir
import concourse.tile as tile
from concourse import bass_utils, masks

B, S, D, N, K = 4, 2048, 1024, 4, 4
EPS = 1e-6
P = 128                 # partitions per feature block
NB = D // P             # 8 feature blocks
T = S // 2              # tokens per core
SUB = 512               # matmul moving-dim tile (one fp32 PSUM bank)
NS = T // SUB           # sub-tiles per core
H = K - 1               # conv halo columns
NPAIR = NB // 2         # DoubleRow contraction pairs for the ctx matmul
F32 = mybir.dt.float32
F32R = mybir.dt.float32r
BF16 = mybir.dt.bfloat16
F8 = mybir.dt.float8e4
OP = mybir.AluOpType
AF = mybir.ActivationFunctionType
DR = mybir.MatmulPerfMode.DoubleRow

_CACHE = {}
ABL = set()          # timing-only ablation flags (test harness use)


def _build(reps=1, no_cc=False):
    nc = bacc.Bacc("TRN2", target_bir_lowering=False, debug=False, num_devices=8)

    # per-core inputs
    xm_d = nc.dram_tensor("xm", [T, D], F32, kind="ExternalInput")
    xhalo_d = nc.dram_tensor("xhalo", [P, NB * H], F32, kind="ExternalInput")
    fbrow_d = nc.dram_tensor("fbrow", [P, D], F32, kind="ExternalInput")
    mask_d = nc.dram_tensor("mask", [P, 1], F32, kind="ExternalInput")
    # packed per-partition aux: [P, NB] / [P, N*NB] with col = i*NB + nb
    cwp_d = nc.dram_tensor("cwp", [P, NB * K], F32, kind="ExternalInput")
    cbp_d = nc.dram_tensor("cbp", [P, NB], F32, kind="ExternalInput")
    gbp_d = nc.dram_tensor("gbp", [P, NB], F32, kind="ExternalInput")
    rwp_d = nc.dram_tensor("rwp", [P, N * NB], F32, kind="ExternalInput")
    abp_d = nc.dram_tensor("abp", [P, N * NB], F32, kind="ExternalInput")
    bbp_d = nc.dram_tensor("bbp", [P, N * NB], F32, kind="ExternalInput")
    ctbp_d = nc.dram_tensor("ctbp", [P, N * NB], F32, kind="ExternalInput")
    frwp_d = nc.dram_tensor("frwp", [P, NB], F32, kind="ExternalInput")
    fbp_d = nc.dram_tensor("fbp", [P, NB], F32, kind="ExternalInput")
    gw_d = nc.dram_tensor("gate_w", [D, D], BF16, kind="ExternalInput")
    aw_d = nc.dram_tensor("alpha_w", [N, D, D], BF16, kind="ExternalInput")
    bw_d = nc.dram_tensor("beta_w", [N, D, D], BF16, kind="ExternalInput")
    # ctx_w: fp8e4, DoubleRow pair-interleaved [N, NPAIR, P, 2, D]
    cw_d = nc.dram_tensor("ctx_w", [N, NPAIR, P, 2, D], F8, kind="ExternalInput")
    fw_d = nc.dram_tensor("fin_w", [D, D], BF16, kind="ExternalInput")
    y_d = nc.dram_tensor("y", [T, D], F32, kind="ExternalOutput")

    with tile.TileContext(nc) as tc:
        _emit(nc, tc, locals(), reps=reps, no_cc=no_cc)
    nc.compile()
    return nc


def _emit(nc, tc, t, reps=1, no_cc=False):
    xm_d = t["xm_d"]; xhalo_d = t["xhalo_d"]; fbrow_d = t["fbrow_d"]
    mask_d = t["mask_d"]; cwp_d = t["cwp_d"]
    cbp_d = t["cbp_d"]; gbp_d = t["gbp_d"]; rwp_d = t["rwp_d"]
    abp_d = t["abp_d"]; bbp_d = t["bbp_d"]; ctbp_d = t["ctbp_d"]
    frwp_d = t["frwp_d"]; fbp_d = t["fbp_d"]; gw_d = t["gw_d"]
    aw_d = t["aw_d"]; bw_d = t["bw_d"]; cw_d = t["cw_d"]; fw_d = t["fw_d"]
    y_d = t["y_d"]

    import contextlib
    with contextlib.ExitStack() as est:
        aux = est.enter_context(tc.tile_pool(name="aux", bufs=1))
        state = est.enter_context(tc.tile_pool(name="state", bufs=1))
        wp = est.enter_context(tc.tile_pool(name="wp", bufs=2))     # bf16 weights
        tmp = est.enter_context(tc.tile_pool(name="tmp", bufs=6))   # [P,SUB] f32
        cnp = est.enter_context(tc.tile_pool(name="cnp", bufs=1))   # cn/fe16/fo16 bf16
        alp = est.enter_context(tc.tile_pool(name="alp", bufs=1))   # alphas f32 (+cacc/po)
        sip = est.enter_context(tc.tile_pool(name="sip", bufs=1))   # sin f32 / xT bf16
        cpp = est.enter_context(tc.tile_pool(name="cpp", bufs=1))   # cumprod bf16
        fe8 = est.enter_context(tc.tile_pool(name="fe8", bufs=1))   # fetched fp8 pairs
        vwp = est.enter_context(tc.tile_pool(name="vwp", bufs=2))   # v bf16 stream
        f32k = est.enter_context(tc.tile_pool(name="f32k", bufs=1))  # [P,1024] f32
        sqr = est.enter_context(tc.tile_pool(name="sqr", bufs=2))   # [P,SUB] f32r
        mmp = est.enter_context(tc.tile_pool(name="mmp", bufs=6, space="PSUM"))
        ssp = est.enter_context(tc.tile_pool(name="ssp", bufs=1, space="PSUM"))
        bcp = est.enter_context(tc.tile_pool(name="bcp", bufs=1, space="PSUM"))
        dram = est.enter_context(tc.tile_pool(name="dram", bufs=1, space="DRAM"))

        # ---- aux constants ----
        def aux_load(name, dram_t, shape):
            # ACT's DMA queue: keeps the SP queue free for the x slabs at start
            tl = aux.tile(shape, F32, name=name)
            nc.scalar.dma_start(tl[:], dram_t[:])
            return tl
        mask = aux_load("mask", mask_d, [P, 1])
        cwp = aux_load("cwp", cwp_d, [P, NB * K])
        cbp = aux_load("cbp", cbp_d, [P, NB])
        gbp = aux_load("gbp", gbp_d, [P, NB])
        rwp = aux_load("rwp", rwp_d, [P, N * NB])
        abp = aux_load("abp", abp_d, [P, N * NB])
        bbp = aux_load("bbp", bbp_d, [P, N * NB])
        ctbp = aux_load("ctbp", ctbp_d, [P, N * NB])
        frwp = aux_load("frwp", frwp_d, [P, NB])
        xhalo = aux_load("xhalo", xhalo_d, [P, NB * H])
        fbrow = aux_load("fbrow", fbrow_d, [P, D])
        ones_f = aux.tile([P, 1], F32)
        nc.vector.memset(ones_f[:], 1.0)
        ones_r = aux.tile([P, 1], F32R)
        nc.vector.tensor_copy(ones_r[:], ones_f[:])
        ones1_f = aux.tile([1, P], F32)
        nc.vector.memset(ones1_f[:], 1.0)
        ones1_r = aux.tile([1, P], F32R)
        nc.vector.tensor_copy(ones1_r[:], ones1_f[:])
        eps_t = aux.tile([P, 1], F32)
        nc.vector.memset(eps_t[:], EPS)
        ident = aux.tile([P, P], F32)
        masks.make_identity(nc, ident[:])
        ident_r = aux.tile([P, P], F32R)
        nc.vector.tensor_copy(ident_r[:], ident[:])

        # ---- DRAM scratch ----
        v_s = [dram.tile([P, T], BF16, name=f"v_s{nb}") for nb in range(NB)]
        og_s = [dram.tile([P, T], BF16, name=f"og_s{nb}") for nb in range(NB)]
        oacc = [dram.tile([P, T], F32, name=f"oacc{nb}") for nb in range(NB)]

        # persistent ctx
        ctxb = [state.tile([P, T], F32, name=f"ctx{nb}") for nb in range(NB)]

        def rms_inv(src, sl, tag_suffix):
            """1/sqrt(mean_d(src^2) + eps) broadcast to [P, SUB].

            src: list of 8 [P, T] f32 tiles. Squares on Pool, reduce and
            broadcast on PE, sqrt on ACT, reciprocal on DVE.
            """
            ssps = ssp.tile([1, SUB], F32, tag="ss", name=f"ss{tag_suffix}")
            for nb in range(NB):
                sq = sqr.tile([P, SUB], F32R, tag="sq", name=f"sq{tag_suffix}_{nb}")
                nc.vector.tensor_tensor(sq[:], src[nb][:, sl], src[nb][:, sl],
                                        OP.mult)
                nc.tensor.matmul(ssps[:], ones_r[:], sq[:],
                                 start=(nb == 0), stop=(nb == NB - 1))
            ssr = sqr.tile([1, SUB], F32R, tag="sq", name=f"ssr{tag_suffix}")
            nc.scalar.copy(ssr[:], ssps[:])
            bc = bcp.tile([P, SUB], F32, tag="bc", name=f"bc{tag_suffix}")
            nc.tensor.matmul(bc[:], ones1_r[:], ssr[:], start=True, stop=True)
            sd = tmp.tile([P, SUB], F32, tag="tmp", name=f"sd{tag_suffix}")
            nc.scalar.activation(sd[:], bc[:], AF.Sqrt, bias=eps_t[:, 0:1],
                                 scale=1.0 / D)
            inv = tmp.tile([P, SUB], F32, tag="tmp", name=f"inv{tag_suffix}")
            nc.vector.reciprocal(inv[:], sd[:])
            return inv

        def load_w(dram_ap, rep, label):
            """Load 8 [P, D] bf16 weight slabs (double-buffered tags)."""
            ws = []
            for k in range(NB):
                wk = wp.tile([P, D], BF16, tag=f"w{k}", name=f"r{rep}_{label}{k}")
                nc.sync.dma_start(wk[:], dram_ap[k * P:(k + 1) * P, :])
                ws.append(wk)
            return ws

        def one_pass(rep):
            # ---- phase 0: x load + PE transpose to [D, T] (bf16), conv, gate
            xT = []
            for nb in range(NB):
                xt = sip.tile([P, H + T], BF16, tag=f"sin{nb}",
                              name=f"r{rep}_xT{nb}")
                nc.vector.tensor_copy(xt[:, 0:H],
                                      xhalo[:, nb * H:(nb + 1) * H])
                xT.append(xt)
            for r in range(4):
                slabs = []
                for j in range(2):
                    tb = r * 2 + j
                    slab = f32k.tile([P, D], F32R, tag=f"b{j}",
                                     name=f"r{rep}_slab{tb}")
                    nc.sync.dma_start(slab[:],
                                      xm_d[tb * P:(tb + 1) * P, :].bitcast(F32R))
                    slabs.append(slab)
                for nb in range(NB):
                    pst = mmp.tile([P, 2 * P], F32R, tag="mm",
                                   name=f"r{rep}_pst{r}_{nb}")
                    for j in range(2):
                        nc.tensor.transpose(
                            pst[:, j * P:(j + 1) * P],
                            slabs[j][:, nb * P:(nb + 1) * P],
                            ident_r[:])
                    nc.scalar.copy(
                        xT[nb][:, H + r * 2 * P:H + (r + 1) * 2 * P], pst[:])

            for nb in range(NB):
                cacc = alp.tile([P, T], F32, tag=f"al{nb}", name=f"r{rep}_cacc{nb}")
                nc.vector.tensor_scalar(
                    cacc[:], xT[nb][:, 0:T], cwp[:, nb * K:nb * K + 1], None,
                    OP.mult)
                for k in range(1, K):
                    nc.vector.scalar_tensor_tensor(
                        cacc[:], xT[nb][:, k:k + T], cwp[:, nb * K + k:nb * K + k + 1],
                        cacc[:], OP.mult, OP.add)
                # c = silu(conv + b) straight into persistent ctx (f32)
                nc.scalar.activation(ctxb[nb][:], cacc[:], AF.Silu,
                                     bias=cbp[:, nb:nb + 1])
                vb = vwp.tile([P, T], BF16, tag="vw", name=f"r{rep}_vb{nb}")
                nc.gpsimd.tensor_copy(vb[:], ctxb[nb][:])
                nc.sync.dma_start(v_s[nb][:], vb[:])
                nc.sync.dma_start(oacc[nb][:], ctxb[nb][:])

            gw = load_w(gw_d, rep, "gw")
            for m in range(NB):
                for s in range(NS):
                    sl = slice(H + s * SUB, H + (s + 1) * SUB)
                    ps = mmp.tile([P, SUB], F32, tag="mm", name=f"r{rep}_psg{m}_{s}")
                    for k in range(NB):
                        nc.tensor.matmul(ps[:], gw[k][:, m * P:(m + 1) * P],
                                         xT[k][:, sl],
                                         start=(k == 0), stop=(k == NB - 1))
                    ogt = tmp.tile([P, SUB], BF16, tag="tmp", name=f"r{rep}_og{m}_{s}")
                    nc.scalar.activation(ogt[:], ps[:], AF.Silu,
                                         bias=gbp[:, m:m + 1])
                    nc.sync.dma_start(og_s[m][:, s * SUB:(s + 1) * SUB], ogt[:])

            # ---- iterations ----
            cn = None
            for i in range(N):
                # R: cn = rmsnorm(ctx) * rms_w[i]   (bf16 out)
                cn = [cnp.tile([P, T], BF16, tag=f"cn{nb}", name=f"r{rep}_cn{i}_{nb}")
                      for nb in range(NB)]
                for s in range(NS):
                    sl = slice(s * SUB, (s + 1) * SUB)
                    inv = rms_inv(ctxb, sl, f"r{i}_{s}")
                    for nb in range(NB):
                        nc.vector.scalar_tensor_tensor(
                            cn[nb][:, sl], ctxb[nb][:, sl],
                            rwp[:, i * NB + nb:i * NB + nb + 1], inv[:],
                            OP.mult, OP.mult)

                # A: alphas = sigmoid(cn @ alpha_w[i] + alpha_b[i])
                wa = load_w(aw_d[i], rep, f"wa{i}_")
                alphas = [alp.tile([P, T], F32, tag=f"al{nb}",
                                   name=f"r{rep}_alphas{i}_{nb}")
                          for nb in range(NB)]
                for m in range(NB):
                    for s in range(NS):
                        sl = slice(s * SUB, (s + 1) * SUB)
                        ps = mmp.tile([P, SUB], F32, tag="mm",
                                      name=f"r{rep}_psa{i}_{m}_{s}")
                        for k in range(NB):
                            nc.tensor.matmul(ps[:], wa[k][:, m * P:(m + 1) * P],
                                             cn[k][:, sl],
                                             start=(k == 0), stop=(k == NB - 1))
                        nc.scalar.activation(alphas[m][:, sl], ps[:], AF.Sigmoid,
                                             bias=abp[:, i * NB + m:i * NB + m + 1])

                # ws = sqrt(1 - alphas^2) into sin; cumprod(alphas) into cp
                sin = [sip.tile([P, T], F32, tag=f"sin{nb}",
                                name=f"r{rep}_sin{i}_{nb}")
                       for nb in range(NB)]
                cp = [cpp.tile([P, T], BF16, tag=f"cp{nb}",
                               name=f"r{rep}_cp{i}_{nb}")
                      for nb in range(NB)]
                for m in range(NB):
                    for s in range(NS):
                        sl = slice(s * SUB, (s + 1) * SUB)
                        asq = tmp.tile([P, SUB], F32, tag="tmp",
                                       name=f"r{rep}_asq{i}_{m}_{s}")
                        nc.gpsimd.tensor_tensor(asq[:], alphas[m][:, sl],
                                                alphas[m][:, sl], OP.mult)
                        nc.scalar.activation(sin[m][:, sl], asq[:], AF.Sqrt,
                                             bias=ones_f[:, 0:1], scale=-1.0)
                    if "noscan" not in ABL:
                        nc.vector.tensor_tensor_scan(
                            cp[m][:], alphas[m][:], alphas[m][:], 1.0,
                            OP.mult, OP.bypass)

                # B: scan_in = silu(cn@beta_w+b) * ws * v, then h-scan per block
                wb = load_w(bw_d[i], rep, f"wb{i}_")
                carries = aux.tile([P, NB], F32, name=f"r{rep}_carries{i}")
                for m in range(NB):
                    vw = vwp.tile([P, T], BF16, tag="vw", name=f"r{rep}_vw{i}_{m}")
                    nc.sync.dma_start(vw[:], v_s[m][:])
                    for s in range(NS):
                        sl = slice(s * SUB, (s + 1) * SUB)
                        ps = mmp.tile([P, SUB], F32, tag="mm",
                                      name=f"r{rep}_psb{i}_{m}_{s}")
                        for k in range(NB):
                            nc.tensor.matmul(ps[:], wb[k][:, m * P:(m + 1) * P],
                                             cn[k][:, sl],
                                             start=(k == 0), stop=(k == NB - 1))
                        bet = tmp.tile([P, SUB], F32, tag="tmp",
                                       name=f"r{rep}_bet{i}_{m}_{s}")
                        nc.scalar.activation(bet[:], ps[:], AF.Silu,
                                             bias=bbp[:, i * NB + m:i * NB + m + 1])
                        # scan_in = (ws * betas) * v, in place over sin (Pool)
                        nc.gpsimd.tensor_tensor(sin[m][:, sl], bet[:],
                                                sin[m][:, sl], OP.mult)
                        nc.gpsimd.tensor_tensor(sin[m][:, sl], sin[m][:, sl],
                                                vw[:, sl], OP.mult)
                    # local scan (initial 0), in place; carry = last column
                    if "noscan" in ABL:
                        nc.vector.tensor_copy(sin[m][:], alphas[m][:])
                    else:
                        nc.vector.tensor_tensor_scan(sin[m][:], alphas[m][:],
                                                     sin[m][:], 0.0,
                                                     OP.mult, OP.add)
                    nc.vector.tensor_copy(carries[:, m:m + 1], sin[m][:, T - 1:T])

                # carry exchange: pair AllGather; c_eff = mask * partner carry
                cin = dram.tile([D], F32, name=f"r{rep}_cin{i}")
                cout = dram.tile([2, D], F32, name=f"r{rep}_cout{i}")
                nc.sync.dma_start(cin[:].rearrange("(p nb) -> p nb", p=P),
                                  carries[:])
                if no_cc:
                    nc.sync.dma_start(cout[0:1, :],
                                      cin[:].rearrange("(a b) -> a b", a=1))
                    nc.sync.dma_start(cout[1:2, :],
                                      cin[:].rearrange("(a b) -> a b", a=1))
                else:
                    nc.gpsimd.collective_compute(
                        "AllGather", OP.bypass,
                        replica_groups=[[0, 1], [2, 3], [4, 5], [6, 7]],
                        ins=[cin.opt()], outs=[cout.opt()])
                gsb = aux.tile([P, NB], F32, name=f"r{rep}_gsb{i}")
                nc.sync.dma_start(
                    gsb[:], cout[0:1, :].rearrange("a (p nb) -> (a p) nb", p=P))
                ceff = aux.tile([P, NB], F32, name=f"r{rep}_ceff{i}")
                nc.vector.tensor_scalar(ceff[:], gsb[:], mask[:, 0:1], None,
                                        OP.mult)

                # fetched = h_local + cumprod * ceff -> fp8 DoubleRow pairs,
                # per subtile with s=0 first so the s-major C matmuls start
                # after ~5us of DVE work.
                fep = [fe8.tile([P, 2, T], F8, tag=f"fe{kp}",
                                name=f"r{rep}_fe{i}_{kp}")
                       for kp in range(NPAIR)]
                for s in range(NS):
                    sl = slice(s * SUB, (s + 1) * SUB)
                    for kp in range(NPAIR):
                        for j in range(2):
                            nb = 2 * kp + j
                            nc.vector.scalar_tensor_tensor(
                                fep[kp][:, j, sl], cp[nb][:, sl],
                                ceff[:, nb:nb + 1], sin[nb][:, sl],
                                OP.mult, OP.add)

                # C: ctx += silu(fetched @ ctx_w[i] + ctx_b[i]), fp8 DoubleRow,
                # s-major so the next iteration's rms can begin on subtile 0
                wc = []
                for kp in range(NPAIR):
                    wck = wp.tile([P, 2, D], F8, tag=f"w{kp}",
                                  name=f"r{rep}_wc{i}_{kp}")
                    nc.sync.dma_start(wck[:], cw_d[i, kp])
                    wc.append(wck)
                for s in range(NS):
                    for m in range(NB):
                        sl = slice(s * SUB, (s + 1) * SUB)
                        ps = mmp.tile([P, SUB], F32, tag="mm",
                                      name=f"r{rep}_psc{i}_{m}_{s}")
                        for kp in range(NPAIR):
                            nc.tensor.matmul(ps[:], wc[kp][:, :, m * P:(m + 1) * P],
                                             fep[kp][:, :, sl],
                                             start=(kp == 0), stop=(kp == NPAIR - 1),
                                             perf_mode=DR)
                        cu = tmp.tile([P, SUB], F32, tag="tmp",
                                      name=f"r{rep}_cu{i}_{m}_{s}")
                        nc.scalar.activation(cu[:], ps[:], AF.Silu,
                                             bias=ctbp[:, i * NB + m:i * NB + m + 1])
                        nc.gpsimd.tensor_tensor(ctxb[m][:, sl], ctxb[m][:, sl],
                                                cu[:], OP.add)
                # out += fetched: exact f32 h part from sin, bf16 correction
                # term; accum-DMA descgen queued after all Pool work
                for nb in range(NB):
                    corr = cnp.tile([P, T], BF16, tag=f"cn{nb}",
                                    name=f"r{rep}_corr{i}_{nb}")
                    nc.gpsimd.tensor_scalar(corr[:], cp[nb][:],
                                            ceff[:, nb:nb + 1], None, OP.mult)
                    nc.gpsimd.dma_start(
                        oacc[nb][:], sin[nb][:],
                        accum_op=OP.bypass if "noaccum" in ABL else OP.add)
                    nc.gpsimd.dma_start(oacc[nb][:], corr[:], accum_op=OP.add)

            # ---- final: y = silu(rmsnorm(out*gate)*fin_rms_w @ fin_w + fin_b)
            po = [alp.tile([P, T], F32, tag=f"al{nb}", name=f"r{rep}_po{nb}")
                  for nb in range(NB)]
            for nb in range(NB):
                ogl = vwp.tile([P, T], BF16, tag="vw", name=f"r{rep}_ogl{nb}")
                nc.sync.dma_start(ogl[:], og_s[nb][:])
                oal = f32k.tile([P, T], F32, tag=f"b{nb % 2}",
                                name=f"r{rep}_oal{nb}")
                nc.sync.dma_start(oal[:], oacc[nb][:])
                nc.vector.tensor_tensor(po[nb][:], oal[:], ogl[:], OP.mult)
            fo = [cnp.tile([P, T], BF16, tag=f"cn{nb}", name=f"r{rep}_fo{nb}")
                  for nb in range(NB)]
            for s in range(NS):
                sl = slice(s * SUB, (s + 1) * SUB)
                inv = rms_inv(po, sl, f"f{s}")
                for nb in range(NB):
                    nc.vector.scalar_tensor_tensor(
                        fo[nb][:, sl], po[nb][:, sl], frwp[:, nb:nb + 1], inv[:],
                        OP.mult, OP.mult)
            fw = load_w(fw_d, rep, "fw")
            for tb in range(NB):
                for do in range(NS):
                    ps = mmp.tile([P, SUB], F32, tag="mm", name=f"r{rep}_psf{tb}_{do}")
                    for k in range(NB):
                        nc.tensor.matmul(ps[:], fo[k][:, tb * P:(tb + 1) * P],
                                         fw[k][:, do * SUB:(do + 1) * SUB],
                                         start=(k == 0), stop=(k == NB - 1))
                    yt = tmp.tile([P, SUB], F32, tag="tmp", name=f"r{rep}_yt{tb}_{do}")
                    nc.vector.tensor_tensor(yt[:], ps[:],
                                            fbrow[:, do * SUB:(do + 1) * SUB],
                                            OP.add)
                    nc.scalar.activation(yt[:], yt[:], AF.Silu)
                    nc.sync.dma_start(
                        y_d[tb * P:(tb + 1) * P, do * SUB:(do + 1) * SUB], yt[:])

        for rep in range(reps):
            one_pass(rep)


def _prep_in_maps(inputs):
    x = np.asarray(inputs["x"], np.float32)
    conv_w = np.asarray(inputs["conv_w"], np.float32)
    conv_b = np.asarray(inputs["conv_b"], np.float32)
    gate_w = np.asarray(inputs["gate_w"], np.float32)
    gate_b = np.asarray(inputs["gate_b"], np.float32)
    rms_w = np.asarray(inputs["rms_w"], np.float32)
    alpha_w = np.asarray(inputs["alpha_w"], np.float32)
    alpha_b = np.asarray(inputs["alpha_b"], np.float32)
    beta_w = np.asarray(inputs["beta_w"], np.float32)
    beta_b = np.asarray(inputs["beta_b"], np.float32)
    ctx_w = np.asarray(inputs["ctx_w"], np.float32)
    ctx_b = np.asarray(inputs["ctx_b"], np.float32)
    fin_rms_w = np.asarray(inputs["fin_rms_w"], np.float32)
    fin_w = np.asarray(inputs["fin_w"], np.float32)
    fin_b = np.asarray(inputs["fin_b"], np.float32)

    def pack1(a):       # [D] -> [P, NB]
        return np.ascontiguousarray(a.reshape(NB, P).T)

    def packN(a):       # [N, D] -> [P, N*NB]
        return np.ascontiguousarray(
            a.reshape(N, NB, P).transpose(2, 0, 1).reshape(P, N * NB))

    def b16(a):
        return np.ascontiguousarray(a.astype(ml_dtypes.bfloat16))

    cwp = np.ascontiguousarray(
        conv_w.T.reshape(NB, P, K).transpose(1, 0, 2).reshape(P, NB * K))
    shared = dict(
        cwp=cwp, cbp=pack1(conv_b), gbp=pack1(gate_b),
        rwp=packN(rms_w), abp=packN(alpha_b), bbp=packN(beta_b),
        ctbp=packN(ctx_b), frwp=pack1(fin_rms_w), fbp=pack1(fin_b),
        gate_w=b16(gate_w),
        alpha_w=b16(alpha_w),
        beta_w=b16(beta_w),
        ctx_w=np.ascontiguousarray(
            ctx_w.reshape(N, NPAIR, 2, P, D).transpose(0, 1, 3, 2, 4)
            .astype(ml_dtypes.float8_e4m3)),
        fin_w=b16(fin_w),
    )
    shared["fbrow"] = np.ascontiguousarray(
        np.broadcast_to(fin_b[None, :], (P, D)))
    in_maps = []
    for c in range(8):
        b, h = c // 2, c % 2
        t0 = h * T
        m = dict(shared)
        m["xm"] = np.ascontiguousarray(x[b, t0:t0 + T])
        if h == 0:
            m["xhalo"] = np.zeros((P, NB * H), np.float32)
        else:
            halo = x[b, t0 - H:t0, :]          # [K-1, D]
            m["xhalo"] = np.ascontiguousarray(
                halo.T.reshape(NB, P, H).transpose(1, 0, 2)
                .reshape(P, NB * H))
        m["mask"] = np.full((P, 1), float(h), np.float32)
        in_maps.append(m)
    return in_maps


def kernel(**inputs) -> np.ndarray:
    if "nc" not in _CACHE:
        _CACHE["nc"] = _build()
    nc = _CACHE["nc"]
    in_maps = _prep_in_maps(inputs)
    res = bass_utils.run_bass_kernel_spmd(nc, in_maps, core_ids=list(range(8)))
    y = np.empty((B, S, D), np.float32)
    for c in range(8):
        b, h = c // 2, c % 2
        y[b, h * T:(h + 1) * T] = res.results[c]["y"]
    return y


# revision 26
# speedup vs baseline: 2.8222x; 2.8222x over previous
"""Trainium2 Bass kernel for nn_Block_343597384085.

Model (per batch b):
  c        = silu(causal_depthwise_conv(x, K=4) + conv_b)
  out_gate = silu(x @ gate_w + gate_b)
  v = ctx = out = c
  for i in 0..3:
      cn      = rmsnorm(ctx) * rms_w[i]
      alphas  = sigmoid(cn @ alpha_w[i] + alpha_b[i])
      betas   = silu(cn @ beta_w[i] + beta_b[i])
      ws      = sqrt(clip(1 - alphas^2, 1e-6))
      fetched = assoc_scan(h_t = a_t h_{t-1} + v_t) over (v*betas*ws, alphas)
      ctx     = ctx + silu(fetched @ ctx_w[i] + ctx_b[i])
      out     = out + fetched
  out = rmsnorm(out * out_gate) * fin_rms_w
  y   = silu(out @ fin_w + fin_b)

Sharding: 8 cores = (batch, seq-half). Each core: 1024 tokens x D=1024,
feature-major SBUF layout [D-block(128 part), tokens(free)].
All matmuls run in bf16 (weights converted on host; 1 cyc/row, same PE
speed as f32r, half the SBUF/DMA). The scan stays fp32. The scan's
cross-half carry moves via a per-iteration pair AllGather (4KB); the
cumprod of alphas is computed BEFORE the carry arrives (initial=1), so
post-carry work is one scalar_tensor_tensor per block.
"""
import numpy as np
import ml_dtypes

import concourse.bass as bass
import concourse.bacc as bacc
import concourse.mybir as mybir
import concourse.tile as tile
from concourse import bass_utils, masks

B, S, D, N, K = 4, 2048, 1024, 4, 4
EPS = 1e-6
P = 128                 # partitions per feature block
NB = D // P             # 8 feature blocks
T = S // 2              # tokens per core
SUB = 512               # matmul moving-dim tile (one fp32 PSUM bank)
NS = T // SUB           # sub-tiles per core
H = K - 1               # conv halo columns
F32 = mybir.dt.float32
F32R = mybir.dt.float32r
BF16 = mybir.dt.bfloat16
OP = mybir.AluOpType
AF = mybir.ActivationFunctionType

_CACHE = {}
ABL = set()          # timing-only ablation flags (test harness use)


def _build(reps=1, no_cc=False):
    nc = bacc.Bacc("TRN2", target_bir_lowering=False, debug=False, num_devices=8)

    # per-core inputs
    xm_d = nc.dram_tensor("xm", [T, D], F32, kind="ExternalInput")
    xhalo_d = nc.dram_tensor("xhalo", [P, NB * H], F32, kind="ExternalInput")
    fbrow_d = nc.dram_tensor("fbrow", [P, D], F32, kind="ExternalInput")
    mask_d = nc.dram_tensor("mask", [P, 1], F32, kind="ExternalInput")
    # packed per-partition aux: [P, NB] / [P, N*NB] with col = i*NB + nb
    cwp_d = nc.dram_tensor("cwp", [P, NB * K], F32, kind="ExternalInput")
    cbp_d = nc.dram_tensor("cbp", [P, NB], F32, kind="ExternalInput")
    gbp_d = nc.dram_tensor("gbp", [P, NB], F32, kind="ExternalInput")
    rwp_d = nc.dram_tensor("rwp", [P, N * NB], F32, kind="ExternalInput")
    abp_d = nc.dram_tensor("abp", [P, N * NB], F32, kind="ExternalInput")
    bbp_d = nc.dram_tensor("bbp", [P, N * NB], F32, kind="ExternalInput")
    ctbp_d = nc.dram_tensor("ctbp", [P, N * NB], F32, kind="ExternalInput")
    frwp_d = nc.dram_tensor("frwp", [P, NB], F32, kind="ExternalInput")
    fbp_d = nc.dram_tensor("fbp", [P, NB], F32, kind="ExternalInput")
    gw_d = nc.dram_tensor("gate_w", [D, D], BF16, kind="ExternalInput")
    aw_d = nc.dram_tensor("alpha_w", [N, D, D], BF16, kind="ExternalInput")
    bw_d = nc.dram_tensor("beta_w", [N, D, D], BF16, kind="ExternalInput")
    cw_d = nc.dram_tensor("ctx_w", [N, D, D], BF16, kind="ExternalInput")
    fw_d = nc.dram_tensor("fin_w", [D, D], BF16, kind="ExternalInput")
    y_d = nc.dram_tensor("y", [T, D], F32, kind="ExternalOutput")

    with tile.TileContext(nc) as tc:
        _emit(nc, tc, locals(), reps=reps, no_cc=no_cc)
    nc.compile()
    return nc


def _emit(nc, tc, t, reps=1, no_cc=False):
    xm_d = t["xm_d"]; xhalo_d = t["xhalo_d"]; fbrow_d = t["fbrow_d"]
    mask_d = t["mask_d"]; cwp_d = t["cwp_d"]
    cbp_d = t["cbp_d"]; gbp_d = t["gbp_d"]; rwp_d = t["rwp_d"]
    abp_d = t["abp_d"]; bbp_d = t["bbp_d"]; ctbp_d = t["ctbp_d"]
    frwp_d = t["frwp_d"]; fbp_d = t["fbp_d"]; gw_d = t["gw_d"]
    aw_d = t["aw_d"]; bw_d = t["bw_d"]; cw_d = t["cw_d"]; fw_d = t["fw_d"]
    y_d = t["y_d"]

    import contextlib
    with contextlib.ExitStack() as est:
        aux = est.enter_context(tc.tile_pool(name="aux", bufs=1))
        state = est.enter_context(tc.tile_pool(name="state", bufs=1))
        wp = est.enter_context(tc.tile_pool(name="wp", bufs=2))     # bf16 weights
        tmp = est.enter_context(tc.tile_pool(name="tmp", bufs=6))   # [P,SUB] f32
        cnp = est.enter_context(tc.tile_pool(name="cnp", bufs=1))   # cn/fe16/fo16 bf16
        alp = est.enter_context(tc.tile_pool(name="alp", bufs=1))   # alphas f32 (+cacc/po)
        sip = est.enter_context(tc.tile_pool(name="sip", bufs=1))   # sin f32 / xT bf16
        cpp = est.enter_context(tc.tile_pool(name="cpp", bufs=1))   # cumprod bf16
        vwp = est.enter_context(tc.tile_pool(name="vwp", bufs=2))   # v bf16 stream
        f32k = est.enter_context(tc.tile_pool(name="f32k", bufs=2))  # [P,1024] f32
        sqr = est.enter_context(tc.tile_pool(name="sqr", bufs=2))   # [P,SUB] f32r
        mmp = est.enter_context(tc.tile_pool(name="mmp", bufs=6, space="PSUM"))
        ssp = est.enter_context(tc.tile_pool(name="ssp", bufs=1, space="PSUM"))
        bcp = est.enter_context(tc.tile_pool(name="bcp", bufs=1, space="PSUM"))
        dram = est.enter_context(tc.tile_pool(name="dram", bufs=1, space="DRAM"))

        # ---- aux constants ----
        def aux_load(name, dram_t, shape):
            # ACT's DMA queue: keeps the SP queue free for the x slabs at start
            tl = aux.tile(shape, F32, name=name)
            nc.scalar.dma_start(tl[:], dram_t[:])
            return tl
        mask = aux_load("mask", mask_d, [P, 1])
        cwp = aux_load("cwp", cwp_d, [P, NB * K])
        cbp = aux_load("cbp", cbp_d, [P, NB])
        gbp = aux_load("gbp", gbp_d, [P, NB])
        rwp = aux_load("rwp", rwp_d, [P, N * NB])
        abp = aux_load("abp", abp_d, [P, N * NB])
        bbp = aux_load("bbp", bbp_d, [P, N * NB])
        ctbp = aux_load("ctbp", ctbp_d, [P, N * NB])
        frwp = aux_load("frwp", frwp_d, [P, NB])
        xhalo = aux_load("xhalo", xhalo_d, [P, NB * H])
        fbrow = aux_load("fbrow", fbrow_d, [P, D])
        ones_f = aux.tile([P, 1], F32)
        nc.vector.memset(ones_f[:], 1.0)
        ones_r = aux.tile([P, 1], F32R)
        nc.vector.tensor_copy(ones_r[:], ones_f[:])
        ones1_f = aux.tile([1, P], F32)
        nc.vector.memset(ones1_f[:], 1.0)
        ones1_r = aux.tile([1, P], F32R)
        nc.vector.tensor_copy(ones1_r[:], ones1_f[:])
        eps_t = aux.tile([P, 1], F32)
        nc.vector.memset(eps_t[:], EPS)
        ident = aux.tile([P, P], F32)
        masks.make_identity(nc, ident[:])
        ident_r = aux.tile([P, P], F32R)
        nc.vector.tensor_copy(ident_r[:], ident[:])

        # ---- DRAM scratch ----
        v_s = [dram.tile([P, T], BF16, name=f"v_s{nb}") for nb in range(NB)]
        og_s = [dram.tile([P, T], BF16, name=f"og_s{nb}") for nb in range(NB)]
        oacc = [dram.tile([P, T], F32, name=f"oacc{nb}") for nb in range(NB)]

        # persistent ctx
        ctxb = [state.tile([P, T], F32, name=f"ctx{nb}") for nb in range(NB)]

        def rms_inv(src, sl, tag_suffix):
            """1/sqrt(mean_d(src^2) + eps) broadcast to [P, SUB].

            src: list of 8 [P, T] f32 tiles. Squares on Pool, reduce and
            broadcast on PE, sqrt on ACT, reciprocal on DVE.
            """
            ssps = ssp.tile([1, SUB], F32, tag="ss", name=f"ss{tag_suffix}")
            for nb in range(NB):
                sq = sqr.tile([P, SUB], F32R, tag="sq", name=f"sq{tag_suffix}_{nb}")
                nc.vector.tensor_tensor(sq[:], src[nb][:, sl], src[nb][:, sl],
                                        OP.mult)
                nc.tensor.matmul(ssps[:], ones_r[:], sq[:],
                                 start=(nb == 0), stop=(nb == NB - 1))
            ssr = sqr.tile([1, SUB], F32R, tag="sq", name=f"ssr{tag_suffix}")
            nc.scalar.copy(ssr[:], ssps[:])
            bc = bcp.tile([P, SUB], F32, tag="bc", name=f"bc{tag_suffix}")
            nc.tensor.matmul(bc[:], ones1_r[:], ssr[:], start=True, stop=True)
            sd = tmp.tile([P, SUB], F32, tag="tmp", name=f"sd{tag_suffix}")
            nc.scalar.activation(sd[:], bc[:], AF.Sqrt, bias=eps_t[:, 0:1],
                                 scale=1.0 / D)
            inv = tmp.tile([P, SUB], F32, tag="tmp", name=f"inv{tag_suffix}")
            nc.vector.reciprocal(inv[:], sd[:])
            return inv

        def load_w(dram_ap, rep, label):
            """Load 8 [P, D] bf16 weight slabs (double-buffered tags)."""
            ws = []
            for k in range(NB):
                wk = wp.tile([P, D], BF16, tag=f"w{k}", name=f"r{rep}_{label}{k}")
                nc.sync.dma_start(wk[:], dram_ap[k * P:(k + 1) * P, :])
                ws.append(wk)
            return ws

        def one_pass(rep):
            # ---- phase 0: x load + PE transpose to [D, T] (bf16), conv, gate
            xT = []
            for nb in range(NB):
                xt = sip.tile([P, H + T], BF16, tag=f"sin{nb}",
                              name=f"r{rep}_xT{nb}")
                nc.vector.tensor_copy(xt[:, 0:H],
                                      xhalo[:, nb * H:(nb + 1) * H])
                xT.append(xt)
            for r in range(2):
                slabs = []
                for j in range(4):
                    tb = r * 4 + j
                    slab = f32k.tile([P, D], F32R, tag=f"b{j % 2}",
                                     name=f"r{rep}_slab{tb}")
                    nc.sync.dma_start(slab[:],
                                      xm_d[tb * P:(tb + 1) * P, :].bitcast(F32R))
                    slabs.append(slab)
                for nb in range(NB):
                    pst = mmp.tile([P, SUB], F32R, tag="mm",
                                   name=f"r{rep}_pst{r}_{nb}")
                    for j in range(4):
                        nc.tensor.transpose(
                            pst[:, j * P:(j + 1) * P],
                            slabs[j][:, nb * P:(nb + 1) * P],
                            ident_r[:])
                    nc.scalar.copy(
                        xT[nb][:, H + r * SUB:H + (r + 1) * SUB], pst[:])

            for nb in range(NB):
                cacc = alp.tile([P, T], F32, tag=f"al{nb}", name=f"r{rep}_cacc{nb}")
                nc.vector.tensor_scalar(
                    cacc[:], xT[nb][:, 0:T], cwp[:, nb * K:nb * K + 1], None,
                    OP.mult)
                for k in range(1, K):
                    nc.vector.scalar_tensor_tensor(
                        cacc[:], xT[nb][:, k:k + T], cwp[:, nb * K + k:nb * K + k + 1],
                        cacc[:], OP.mult, OP.add)
                # c = silu(conv + b) straight into persistent ctx (f32)
                nc.scalar.activation(ctxb[nb][:], cacc[:], AF.Silu,
                                     bias=cbp[:, nb:nb + 1])
                vb = vwp.tile([P, T], BF16, tag="vw", name=f"r{rep}_vb{nb}")
                nc.gpsimd.tensor_copy(vb[:], ctxb[nb][:])
                nc.sync.dma_start(v_s[nb][:], vb[:])
                nc.sync.dma_start(oacc[nb][:], ctxb[nb][:])

            gw = load_w(gw_d, rep, "gw")
            for m in range(NB):
                for s in range(NS):
                    sl = slice(H + s * SUB, H + (s + 1) * SUB)
                    ps = mmp.tile([P, SUB], F32, tag="mm", name=f"r{rep}_psg{m}_{s}")
                    for k in range(NB):
                        nc.tensor.matmul(ps[:], gw[k][:, m * P:(m + 1) * P],
                                         xT[k][:, sl],
                                         start=(k == 0), stop=(k == NB - 1))
                    ogt = tmp.tile([P, SUB], BF16, tag="tmp", name=f"r{rep}_og{m}_{s}")
                    nc.scalar.activation(ogt[:], ps[:], AF.Silu,
                                         bias=gbp[:, m:m + 1])
                    nc.sync.dma_start(og_s[m][:, s * SUB:(s + 1) * SUB], ogt[:])

            # ---- iterations ----
            cn = None
            for i in range(N):
                # R: cn = rmsnorm(ctx) * rms_w[i]   (bf16 out)
                cn = [cnp.tile([P, T], BF16, tag=f"cn{nb}", name=f"r{rep}_cn{i}_{nb}")
                      for nb in range(NB)]
                for s in range(NS):
                    sl = slice(s * SUB, (s + 1) * SUB)
                    inv = rms_inv(ctxb, sl, f"r{i}_{s}")
                    for nb in range(NB):
                        nc.vector.scalar_tensor_tensor(
                            cn[nb][:, sl], ctxb[nb][:, sl],
                            rwp[:, i * NB + nb:i * NB + nb + 1], inv[:],
                            OP.mult, OP.mult)

                # A: alphas = sigmoid(cn @ alpha_w[i] + alpha_b[i])
                wa = load_w(aw_d[i], rep, f"wa{i}_")
                alphas = [alp.tile([P, T], F32, tag=f"al{nb}",
                                   name=f"r{rep}_alphas{i}_{nb}")
                          for nb in range(NB)]
                for m in range(NB):
                    for s in range(NS):
                        sl = slice(s * SUB, (s + 1) * SUB)
                        ps = mmp.tile([P, SUB], F32, tag="mm",
                                      name=f"r{rep}_psa{i}_{m}_{s}")
                        for k in range(NB):
                            nc.tensor.matmul(ps[:], wa[k][:, m * P:(m + 1) * P],
                                             cn[k][:, sl],
                                             start=(k == 0), stop=(k == NB - 1))
                        nc.scalar.activation(alphas[m][:, sl], ps[:], AF.Sigmoid,
                                             bias=abp[:, i * NB + m:i * NB + m + 1])

                # ws = sqrt(1 - alphas^2) into sin; cumprod(alphas) into cp
                sin = [sip.tile([P, T], F32, tag=f"sin{nb}",
                                name=f"r{rep}_sin{i}_{nb}")
                       for nb in range(NB)]
                cp = [cpp.tile([P, T], BF16, tag=f"cp{nb}",
                               name=f"r{rep}_cp{i}_{nb}")
                      for nb in range(NB)]
                for m in range(NB):
                    for s in range(NS):
                        sl = slice(s * SUB, (s + 1) * SUB)
                        asq = tmp.tile([P, SUB], F32, tag="tmp",
                                       name=f"r{rep}_asq{i}_{m}_{s}")
                        nc.gpsimd.tensor_tensor(asq[:], alphas[m][:, sl],
                                                alphas[m][:, sl], OP.mult)
                        nc.scalar.activation(sin[m][:, sl], asq[:], AF.Sqrt,
                                             bias=ones_f[:, 0:1], scale=-1.0)
                    if "noscan" not in ABL:
                        nc.vector.tensor_tensor_scan(
                            cp[m][:], alphas[m][:], alphas[m][:], 1.0,
                            OP.mult, OP.bypass)

                # B: scan_in = silu(cn@beta_w+b) * ws * v, then h-scan per block
                wb = load_w(bw_d[i], rep, f"wb{i}_")
                carries = aux.tile([P, NB], F32, name=f"r{rep}_carries{i}")
                for m in range(NB):
                    vw = vwp.tile([P, T], BF16, tag="vw", name=f"r{rep}_vw{i}_{m}")
                    nc.sync.dma_start(vw[:], v_s[m][:])
                    for s in range(NS):
                        sl = slice(s * SUB, (s + 1) * SUB)
                        ps = mmp.tile([P, SUB], F32, tag="mm",
                                      name=f"r{rep}_psb{i}_{m}_{s}")
                        for k in range(NB):
                            nc.tensor.matmul(ps[:], wb[k][:, m * P:(m + 1) * P],
                                             cn[k][:, sl],
                                             start=(k == 0), stop=(k == NB - 1))
                        bet = tmp.tile([P, SUB], F32, tag="tmp",
                                       name=f"r{rep}_bet{i}_{m}_{s}")
                        nc.scalar.activation(bet[:], ps[:], AF.Silu,
                                             bias=bbp[:, i * NB + m:i * NB + m + 1])
                        # scan_in = (ws * betas) * v, in place over sin (Pool)
                        nc.gpsimd.tensor_tensor(sin[m][:, sl], bet[:],
                                                sin[m][:, sl], OP.mult)
                        nc.gpsimd.tensor_tensor(sin[m][:, sl], sin[m][:, sl],
                                                vw[:, sl], OP.mult)
                    # local scan (initial 0), in place; carry = last column
                    if "noscan" in ABL:
                        nc.vector.tensor_copy(sin[m][:], alphas[m][:])
                    else:
                        nc.vector.tensor_tensor_scan(sin[m][:], alphas[m][:],
                                                     sin[m][:], 0.0,
                                                     OP.mult, OP.add)
                    nc.vector.tensor_copy(carries[:, m:m + 1], sin[m][:, T - 1:T])

                # carry exchange: pair AllGather; c_eff = mask * partner carry
                cin = dram.tile([D], F32, name=f"r{rep}_cin{i}")
                cout = dram.tile([2, D], F32, name=f"r{rep}_cout{i}")
                nc.sync.dma_start(cin[:].rearrange("(p nb) -> p nb", p=P),
                                  carries[:])
                if no_cc:
                    nc.sync.dma_start(cout[0:1, :],
                                      cin[:].rearrange("(a b) -> a b", a=1))
                    nc.sync.dma_start(cout[1:2, :],
                                      cin[:].rearrange("(a b) -> a b", a=1))
                else:
                    nc.gpsimd.collective_compute(
                        "AllGather", OP.bypass,
                        replica_groups=[[0, 1], [2, 3], [4, 5], [6, 7]],
                        ins=[cin.opt()], outs=[cout.opt()])
                gsb = aux.tile([P, NB], F32, name=f"r{rep}_gsb{i}")
                nc.sync.dma_start(
                    gsb[:], cout[0:1, :].rearrange("a (p nb) -> (a p) nb", p=P))
                ceff = aux.tile([P, NB], F32, name=f"r{rep}_ceff{i}")
                nc.vector.tensor_scalar(ceff[:], gsb[:], mask[:, 0:1], None,
                                        OP.mult)

                # fetched = h_local + cumprod * ceff -> bf16, per subtile with
                # s=0 first so the s-major C matmuls start after ~5us of DVE
                # work. oacc accumulates the bf16 fetched via dtype-converting
                # accum-DMA at the end of the iteration (no f32 copy needed).
                fe16 = [cnp.tile([P, T], BF16, tag=f"cn{nb}",
                                 name=f"r{rep}_fe{i}_{nb}")
                        for nb in range(NB)]
                for s in range(NS):
                    sl = slice(s * SUB, (s + 1) * SUB)
                    for nb in range(NB):
                        nc.vector.scalar_tensor_tensor(
                            fe16[nb][:, sl], cp[nb][:, sl], ceff[:, nb:nb + 1],
                            sin[nb][:, sl], OP.mult, OP.add)

                # C: ctx += silu(fetched @ ctx_w[i] + ctx_b[i]), s-major so the
                # next iteration's rms can begin on subtile 0 early
                wc = load_w(cw_d[i], rep, f"wc{i}_")
                for s in range(NS):
                    for m in range(NB):
                        sl = slice(s * SUB, (s + 1) * SUB)
                        ps = mmp.tile([P, SUB], F32, tag="mm",
                                      name=f"r{rep}_psc{i}_{m}_{s}")
                        for k in range(NB):
                            nc.tensor.matmul(ps[:], wc[k][:, m * P:(m + 1) * P],
                                             fe16[k][:, sl],
                                             start=(k == 0), stop=(k == NB - 1))
                        cu = tmp.tile([P, SUB], F32, tag="tmp",
                                      name=f"r{rep}_cu{i}_{m}_{s}")
                        nc.scalar.activation(cu[:], ps[:], AF.Silu,
                                             bias=ctbp[:, i * NB + m:i * NB + m + 1])
                        nc.gpsimd.tensor_tensor(ctxb[m][:, sl], ctxb[m][:, sl],
                                                cu[:], OP.add)
                # out += fetched (accum-DMA descgen queued after all Pool work)
                for nb in range(NB):
                    nc.gpsimd.dma_start(
                        oacc[nb][:], fe16[nb][:],
                        accum_op=OP.bypass if "noaccum" in ABL else OP.add)

            # ---- final: y = silu(rmsnorm(out*gate)*fin_rms_w @ fin_w + fin_b)
            po = [alp.tile([P, T], F32, tag=f"al{nb}", name=f"r{rep}_po{nb}")
                  for nb in range(NB)]
            for nb in range(NB):
                ogl = vwp.tile([P, T], BF16, tag="vw", name=f"r{rep}_ogl{nb}")
                nc.sync.dma_start(ogl[:], og_s[nb][:])
                oal = f32k.tile([P, T], F32, tag=f"b{nb % 2}",
                                name=f"r{rep}_oal{nb}")
                nc.sync.dma_start(oal[:], oacc[nb][:])
                nc.vector.tensor_tensor(po[nb][:], oal[:], ogl[:], OP.mult)
            fo = [cnp.tile([P, T], BF16, tag=f"cn{nb}", name=f"r{rep}_fo{nb}")
                  for nb in range(NB)]
            for s in range(NS):
                sl = slice(s * SUB, (s + 1) * SUB)
                inv = rms_inv(po, sl, f"f{s}")
                for nb in range(NB):
                    nc.vector.scalar_tensor_tensor(
                        fo[nb][:, sl], po[nb][:, sl], frwp[:, nb:nb + 1], inv[:],
                        OP.mult, OP.mult)
            fw = load_w(fw_d, rep, "fw")
            for tb in range(NB):
                for do in range(NS):
                    ps = mmp.tile([P, SUB], F32, tag="mm", name=f"r{rep}_psf{tb}_{do}")
                    for k in range(NB):
                        nc.tensor.matmul(ps[:], fo[k][:, tb * P:(tb + 1) * P],
                                         fw[k][:, do * SUB:(do + 1) * SUB],
                                         start=(k == 0), stop=(k == NB - 1))
                    yt = tmp.tile([P, SUB], F32, tag="tmp", name=f"r{rep}_yt{tb}_{do}")
                    nc.vector.tensor_tensor(yt[:], ps[:],
                                            fbrow[:, do * SUB:(do + 1) * SUB],
                                            OP.add)
                    nc.scalar.activation(yt[:], yt[:], AF.Silu)
                    nc.sync.dma_start(
                        y_d[tb * P:(tb + 1) * P, do * SUB:(do + 1) * SUB], yt[:])

        for rep in range(reps):
            one_pass(rep)


def _prep_in_maps(inputs):
    x = np.asarray(inputs["x"], np.float32)
    conv_w = np.asarray(inputs["conv_w"], np.float32)
    conv_b = np.asarray(inputs["conv_b"], np.float32)
    gate_w = np.asarray(inputs["gate_w"], np.float32)
    gate_b = np.asarray(inputs["gate_b"], np.float32)
    rms_w = np.asarray(inputs["rms_w"], np.float32)
    alpha_w = np.asarray(inputs["alpha_w"], np.float32)
    alpha_b = np.asarray(inputs["alpha_b"], np.float32)
    beta_w = np.asarray(inputs["beta_w"], np.float32)
    beta_b = np.asarray(inputs["beta_b"], np.float32)
    ctx_w = np.asarray(inputs["ctx_w"], np.float32)
    ctx_b = np.asarray(inputs["ctx_b"], np.float32)
    fin_rms_w = np.asarray(inputs["fin_rms_w"], np.float32)
    fin_w = np.asarray(inputs["fin_w"], np.float32)
    fin_b = np.asarray(inputs["fin_b"], np.float32)

    def pack1(a):       # [D] -> [P, NB]
        return np.ascontiguousarray(a.reshape(NB, P).T)

    def packN(a):       # [N, D] -> [P, N*NB]
        return np.ascontiguousarray(
            a.reshape(N, NB, P).transpose(2, 0, 1).reshape(P, N * NB))

    def b16(a):
        return np.ascontiguousarray(a.astype(ml_dtypes.bfloat16))

    cwp = np.ascontiguousarray(
        conv_w.T.reshape(NB, P, K).transpose(1, 0, 2).reshape(P, NB * K))
    shared = dict(
        cwp=cwp, cbp=pack1(conv_b), gbp=pack1(gate_b),
        rwp=packN(rms_w), abp=packN(alpha_b), bbp=packN(beta_b),
        ctbp=packN(ctx_b), frwp=pack1(fin_rms_w), fbp=pack1(fin_b),
        gate_w=b16(gate_w),
        alpha_w=b16(alpha_w),
        beta_w=b16(beta_w),
        ctx_w=b16(ctx_w),
        fin_w=b16(fin_w),
    )
    shared["fbrow"] = np.ascontiguousarray(
        np.broadcast_to(fin_b[None, :], (P, D)))
    in_maps = []
    for c in range(8):
        b, h = c // 2, c % 2
        t0 = h * T
        m = dict(shared)
        m["xm"] = np.ascontiguousarray(x[b, t0:t0 + T])
        if h == 0:
            m["xhalo"] = np.zeros((P, NB * H), np.float32)
        else:
            halo = x[b, t0 - H:t0, :]          # [K-1, D]
            m["xhalo"] = np.ascontiguousarray(
                halo.T.reshape(NB, P, H).transpose(1, 0, 2)
                .reshape(P, NB * H))
        m["mask"] = np.full((P, 1), float(h), np.float32)
        in_maps.append(m)
    return in_maps


def kernel(**inputs) -> np.ndarray:
    if "nc" not in _CACHE:
        _CACHE["nc"] = _build()
    nc = _CACHE["nc"]
    in_maps = _prep_in_maps(inputs)
    res = bass_utils.run_bass_kernel_spmd(nc, in_maps, core_ids=list(range(8)))
    y = np.empty((B, S, D), np.float32)
    for c in range(8):
        b, h = c // 2, c % 2
        y[b, h * T:(h + 1) * T] = res.results[c]["y"]
    return y
